# revision 1
# baseline (speedup 1.0000x reference)
"""Trainium2 Bass kernel for: 1x1-conv GEMM + GroupNorm + HardTanh.

Reference computation (per sample b):
    y = weight @ x[b]                        # [512, 256] @ [256, 56*56]
    groupnorm over 32 groups of 16 channels  # stats over (16, 56*56)
    y = y * gamma + beta                     # per-channel affine
    out = clip(y, -2, 2)

Sharding: data-parallel over batch, 4 samples per core x 8 cores.
weight/gamma/beta replicated. No cross-core communication needed.

Matmul runs in float32r (full PE rate for N>=256, ~1e-3 accuracy vs
4x slower plain fp32; measured output error 4.3e-4 of scale).
Per-partition GroupNorm stats come from bn_stats/bn_aggr reading PSUM
directly; the 16-partition group reduction AND broadcast back to all
partitions is ONE tiny PE matmul against a block-diagonal 1/16 matrix,
written into the spare tail columns of the last matmul tile's PSUM
bank (so all 8 banks stay available to matmul tiles and the next
chunk's matmuls overlap the current chunk's normalization chain).
Engine assignment per 128-channel chunk: PE matmuls -> DVE bn_stats
(from PSUM) -> tiny DVE/ACT chain for rstd/scale/bias -> ACT affine
per tile (frees PSUM banks incrementally) -> Pool clamp -> store.
x loads are split into column-range quarters and prefetched two
samples deep so the SP DMA FIFO never starves the matmuls.
"""

import sys

sys.path.insert(0, "/opt/trn_rl_repo")

import numpy as np

import concourse.bacc as bacc
import concourse.mybir as mybir
import concourse.tile as tile
from concourse.bass_utils import run_bass_kernel_spmd

# Problem shape (hardcoded per contest contract)
B, CIN, COUT, H, W = 32, 256, 512, 56, 56
HW = H * W  # 3136
G = 32  # num groups
GSIZE = COUT // G  # 16 channels per group
EPS = 1e-5
HT_MIN, HT_MAX = -2.0, 2.0

N_CORES = 8
BPC = B // N_CORES  # samples per core = 4
KC = CIN // 128  # contraction chunks = 2
OC = COUT // 128  # output-channel chunks = 4
NT = 7  # free-dim tiles per row
NTS = HW // NT  # 448 per tile (one PSUM bank, fp32)

_NC_CACHE = None


def _build_program():
    f32 = mybir.dt.float32
    f32r = mybir.dt.float32r

    nc = bacc.Bacc("TRN2", target_bir_lowering=False, debug=False)

    x_d = nc.dram_tensor("x", [BPC, CIN, HW], f32r, kind="ExternalInput")
    wt_d = nc.dram_tensor("wt", [CIN, COUT], f32r, kind="ExternalInput")
    gamma_d = nc.dram_tensor("gamma", [COUT], f32, kind="ExternalInput")
    beta_d = nc.dram_tensor("beta", [COUT], f32, kind="ExternalInput")
    agg_d = nc.dram_tensor("agg", [128, 128], f32, kind="ExternalInput")
    out_d = nc.dram_tensor("out", [BPC, COUT, HW], f32, kind="ExternalOutput")

    with tile.TileContext(nc) as tc:
        with (
            tc.tile_pool(name="singles", bufs=1) as singles,
            tc.tile_pool(name="xp", bufs=4) as xp,
            tc.tile_pool(name="op", bufs=3) as op,
            tc.tile_pool(name="small", bufs=4) as small,
            tc.tile_pool(name="psy", bufs=8, space="PSUM") as psy,
        ):
            # --- one-time setup -------------------------------------------
            # first sample's first x quarter goes FIRST on HWDGE so the
            # first matmul can start ASAP; scalars ride SWDGE (gpsimd)
            XQ = 4  # x loaded in 4 column-range DMAs so matmuls start early
            QW = HW // XQ  # 784
            x0_sb = xp.tile([128, KC, HW], f32r, tag="x")
            nc.sync.dma_start(
                out=x0_sb[:, :, 0:QW],
                in_=x_d.ap()[0, :, 0:QW].rearrange("(c p) f -> p c f", p=128),
            )
            wt_sb = singles.tile([128, KC, COUT], f32r)
            nc.sync.dma_start(
                out=wt_sb, in_=wt_d.ap().rearrange("(c p) m -> p c m", p=128)
            )
            gamma_sb = singles.tile([128, OC], f32)
            nc.gpsimd.dma_start(
                out=gamma_sb, in_=gamma_d.ap().rearrange("(c p) -> p c", p=128)
            )
            beta_sb = singles.tile([128, OC], f32)
            nc.gpsimd.dma_start(
                out=beta_sb, in_=beta_d.ap().rearrange("(c p) -> p c", p=128)
            )
            eps_sb = singles.tile([128, 1], f32)
            nc.vector.memset(eps_sb, EPS)
            agg_sb = singles.tile([128, 128], f32)
            nc.gpsimd.dma_start(out=agg_sb, in_=agg_d.ap())

            # --- main loop ------------------------------------------------
            def load_x_quarter(x_tile, b, q):
                qsl = slice(q * QW, (q + 1) * QW)
                nc.sync.dma_start(
                    out=x_tile[:, :, qsl],
                    in_=x_d.ap()[b, :, qsl].rearrange("(c p) f -> p c f", p=128),
                )

            x_tiles = [x0_sb]
            for q in range(1, XQ):
                load_x_quarter(x0_sb, 0, q)

            for b in range(BPC):
                x_sb = x_tiles[b]
                for oc in range(OC):
                    # spread next sample's x-load quarters between chunks so
                    # they enter the SP DMA FIFO ahead of later stores
                    if b + 1 < BPC and oc < 2:
                        if oc == 0:
                            xnext = xp.tile([128, KC, HW], f32r, tag="x")
                            x_tiles.append(xnext)
                        for j in range(XQ // 2):
                            load_x_quarter(
                                x_tiles[b + 1], b + 1, (XQ // 2) * oc + j
                            )
                    osl = slice(oc * 128, (oc + 1) * 128)
                    st = small.tile([128, NT, 6], f32, tag="st")

                    ps_tiles = []
                    for nt in range(NT):
                        nsl = slice(nt * NTS, (nt + 1) * NTS)
                        ps = psy.tile([128, 512], f32, tag="ymm")
                        ps_tiles.append(ps)
                        for c in range(KC):
                            nc.tensor.matmul(
                                ps[:, 0:NTS],
                                wt_sb[:, c, osl],
                                x_sb[:, c, nsl],
                                start=(c == 0),
                                stop=(c == KC - 1),
                            )
                        nc.vector.bn_stats(out=st[:, nt, :], in_=ps[:, 0:NTS])

                    # per-partition stats: stat3 = [mean, var, mean^2]
                    stat3 = small.tile([128, 3], f32, tag="stat3")
                    nc.vector.bn_aggr(out=stat3[:, 0:2], in_=st)
                    nc.vector.tensor_mul(stat3[:, 2:3], stat3[:, 0:1], stat3[:, 0:1])

                    # group-aggregate + broadcast in one matmul, written into
                    # the unused tail columns of the LAST tile's PSUM bank
                    # (that bank lives longest anyway):
                    # gps[p, j] = avg over p' in group(p) of stat3[p', j]
                    gps = ps_tiles[NT - 1][:, NTS : NTS + 3]
                    nc.tensor.matmul(
                        gps, agg_sb, stat3, start=True, stop=True,
                        skip_group_check=True,
                    )
                    gs = small.tile([128, 3], f32, tag="gs")
                    nc.vector.tensor_copy(out=gs, in_=gps)

                    # group var = E[var] + E[m^2] - mean_g^2
                    # sd = sqrt(var_g + eps); rstd = 1/sd
                    msq = small.tile([128, 1], f32, tag="msq")
                    nc.vector.tensor_mul(msq, gs[:, 0:1], gs[:, 0:1])
                    sd = small.tile([128, 1], f32, tag="sd")
                    nc.vector.tensor_scalar(
                        out=sd,
                        in0=gs[:, 1:2],
                        scalar1=gs[:, 2:3],
                        scalar2=msq,
                        op0=mybir.AluOpType.add,
                        op1=mybir.AluOpType.subtract,
                    )
                    nc.scalar.activation(
                        out=sd,
                        in_=sd,
                        func=mybir.ActivationFunctionType.Sqrt,
                        bias=eps_sb,
                    )
                    rstd = small.tile([128, 1], f32, tag="rstd")
                    nc.vector.reciprocal(rstd, sd)

                    # s = rstd*gamma ; bv = beta - mean*s
                    s = small.tile([128, 1], f32, tag="s")
                    nc.vector.tensor_mul(s, rstd, gamma_sb[:, oc : oc + 1])
                    ms = small.tile([128, 1], f32, tag="ms")
                    nc.vector.tensor_mul(ms, gs[:, 0:1], s)
                    bv = small.tile([128, 1], f32, tag="bv")
                    nc.vector.tensor_sub(bv, beta_sb[:, oc : oc + 1], ms)

                    # yn = y*s + bv per tile (ACT, reads PSUM, frees banks
                    # incrementally); clamp on Pool; store pairs
                    yn_sb = op.tile([128, HW], f32, tag="yn")
                    for nt in range(NT):
                        nsl = slice(nt * NTS, (nt + 1) * NTS)
                        nc.scalar.activation(
                            out=yn_sb[:, nsl],
                            in_=ps_tiles[nt][:, 0:NTS],
                            func=mybir.ActivationFunctionType.Identity,
                            bias=bv,
                            scale=s,
                        )
                        last_chunk = b == BPC - 1 and oc == OC - 1
                        if last_chunk:
                            # drain the final chunk per tile so the last
                            # store finishes right after the last affine
                            flush = [(nt, nt * NTS)]
                        elif nt in (1, 3, 5, NT - 1):
                            flush = [(nt, {1: 0, 3: 2, 5: 4, NT - 1: 6}[nt] * NTS)]
                        else:
                            flush = []
                        for _nt, lo in flush:
                            hsl = slice(lo, (_nt + 1) * NTS)
                            nc.gpsimd.tensor_scalar(
                                out=yn_sb[:, hsl],
                                in0=yn_sb[:, hsl],
                                scalar1=HT_MAX,
                                scalar2=HT_MIN,
                                op0=mybir.AluOpType.min,
                                op1=mybir.AluOpType.max,
                            )
                            nc.sync.dma_start(
                                out=out_d.ap()[b, osl, hsl], in_=yn_sb[:, hsl]
                            )

    nc.compile()
    return nc


def _get_program():
    global _NC_CACHE
    if _NC_CACHE is None:
        _NC_CACHE = _build_program()
    return _NC_CACHE


def _make_in_maps(x, weight, gamma, beta):
    xr = np.ascontiguousarray(x.reshape(B, CIN, HW))
    wt = np.ascontiguousarray(weight.T)  # [CIN, COUT]
    gamma = np.ascontiguousarray(gamma, dtype=np.float32)
    beta = np.ascontiguousarray(beta, dtype=np.float32)
    agg = np.zeros((128, 128), dtype=np.float32)
    for g in range(128 // GSIZE):
        agg[g * GSIZE : (g + 1) * GSIZE, g * GSIZE : (g + 1) * GSIZE] = 1.0 / GSIZE
    return [
        {
            "x": xr[i * BPC : (i + 1) * BPC],
            "wt": wt,
            "gamma": gamma,
            "beta": beta,
            "agg": agg,
        }
        for i in range(N_CORES)
    ]


def kernel(x, weight, gamma, beta):
    x = np.asarray(x, dtype=np.float32)
    weight = np.asarray(weight, dtype=np.float32)
    assert x.shape == (B, CIN, H, W)
    nc = _get_program()
    in_maps = _make_in_maps(x, weight, gamma, beta)
    res = run_bass_kernel_spmd(nc, in_maps, core_ids=list(range(N_CORES)))
    out = np.concatenate([r["out"] for r in res.results], axis=0)
    return out.reshape(B, COUT, H, W)



# revision 3
# speedup vs baseline: 1.1653x; 1.1653x over previous
"""Trainium2 Bass kernel for: 1x1-conv GEMM + GroupNorm + HardTanh.

Reference computation (per sample b):
    y = weight @ x[b]                        # [512, 256] @ [256, 56*56]
    groupnorm over 32 groups of 16 channels  # stats over (16, 56*56)
    y = y * gamma + beta                     # per-channel affine
    out = clip(y, -2, 2)

Sharding: data-parallel over batch, 4 samples per core x 8 cores.
weight/gamma/beta replicated. No cross-core communication needed.

This version is HBM-bandwidth-bound by design: x and the output travel
as fp16 (halving DMA traffic vs fp32; fp16's 10 mantissa bits keep the
end-to-end error ~1e-3 of scale) and the matmul runs in fp16 at the
full PE rate. Engine assignment per 128-channel chunk:
  PE   : 512-col matmuls into two 4-bank PSUM tiles (2 rotating slots,
         so chunk k+1 fills while chunk k drains), plus one tiny
         group-aggregation matmul per sample.
  ACT  : the only PSUM reader - Copy PSUM fp32 -> SBUF fp16, with
         accum_out giving per-channel sum(y) as a free side effect.
  Pool : squares the head columns (TensorTensor mult, fp16).
  DVE  : squares the tail columns (TT mult, 2x mode), then in 4x mode:
         sum(y^2) via tensor_scalar(mult, add-accum), the affine
         (mult s, add bv), and the full clamp (min 2, max -2).
Group stats are per-SAMPLE: the 12 per-chunk accumulators feed one tiny
PE matmul against a replicated block-diagonal averaging matrix
(group-reduce + broadcast in one shot), then a short DVE/ACT chain
yields per-channel scale/bias for the affine.
"""

import sys

sys.path.insert(0, "/opt/trn_rl_repo")

import numpy as np

import concourse.bacc as bacc
import concourse.mybir as mybir
import concourse.tile as tile
from concourse.bass_utils import run_bass_kernel_spmd

# Problem shape (hardcoded per contest contract)
B, CIN, COUT, H, W = 32, 256, 512, 56, 56
HW = H * W  # 3136
G = 32  # num groups
GSIZE = COUT // G  # 16 channels per group
EPS = 1e-5
HT_MIN, HT_MAX = -2.0, 2.0

N_CORES = 8
BPC = B // N_CORES  # samples per core = 4
KC = CIN // 128  # contraction chunks = 2
OC = COUT // 128  # output-channel chunks = 4

HWA = 2048  # first PSUM tile covers hw cols [0, 2048)
HWB = HW - HWA  # second covers [2048, 3136) = 1088 cols
# hw tiling inside the two PSUM tiles: 512-col matmuls (one fp32 bank)
A_TILES = [(t * 512, 512) for t in range(4)]
B_TILES = [(0, 512), (512, 512), (1024, 64)]
GPS_OFF = 1536  # group-stats scratch cols inside the last B psum tile

# square-pass column split: gpsimd takes the head, DVE (2x mode) the tail
TP_COLS = 1760

XQ = 4  # x loaded in 4 column-range DMAs so matmuls start early
QW = HW // XQ  # 784

_NC_CACHE = None


def _build_program():
    f32 = mybir.dt.float32
    f16 = mybir.dt.float16

    nc = bacc.Bacc("TRN2", target_bir_lowering=False, debug=False)

    x_d = nc.dram_tensor("x", [BPC, CIN, HW], f16, kind="ExternalInput")
    wt_d = nc.dram_tensor("wt", [CIN, COUT], f16, kind="ExternalInput")
    gamma_d = nc.dram_tensor("gamma", [COUT], f32, kind="ExternalInput")
    beta_d = nc.dram_tensor("beta", [COUT], f32, kind="ExternalInput")
    agg_d = nc.dram_tensor("agg", [128, 128], f32, kind="ExternalInput")
    out_d = nc.dram_tensor("out", [BPC, COUT, HW], f16, kind="ExternalOutput")

    with tile.TileContext(nc) as tc:
        with (
            tc.tile_pool(name="singles", bufs=1) as singles,
            tc.tile_pool(name="xp", bufs=2) as xp,
            tc.tile_pool(name="yp", bufs=8) as yp,
            tc.tile_pool(name="up", bufs=3) as up,
            tc.tile_pool(name="fp", bufs=4) as fp,
            tc.tile_pool(name="tp", bufs=2) as tp,
            tc.tile_pool(name="small", bufs=2) as small,
            tc.tile_pool(name="psy", bufs=2, space="PSUM") as psy,
        ):
            # --- one-time setup -------------------------------------------
            # first sample's first x quarter goes FIRST on HWDGE so the
            # first matmul can start ASAP; scalars ride SWDGE (gpsimd)
            x0_sb = xp.tile([128, KC, HW], f16, tag="x")
            nc.sync.dma_start(
                out=x0_sb[:, :, 0:QW],
                in_=x_d.ap()[0, :, 0:QW].rearrange("(c p) f -> p c f", p=128),
            )
            wt_sb = singles.tile([128, KC, COUT], f16)
            nc.sync.dma_start(
                out=wt_sb, in_=wt_d.ap().rearrange("(c p) m -> p c m", p=128)
            )
            gamma_sb = singles.tile([128, OC], f32)
            nc.gpsimd.dma_start(
                out=gamma_sb, in_=gamma_d.ap().rearrange("(c p) -> p c", p=128)
            )
            beta_sb = singles.tile([128, OC], f32)
            nc.gpsimd.dma_start(
                out=beta_sb, in_=beta_d.ap().rearrange("(c p) -> p c", p=128)
            )
            eps_sb = singles.tile([128, 1], f32)
            nc.vector.memset(eps_sb, EPS)
            agg_sb = singles.tile([128, 128], f32)
            nc.gpsimd.dma_start(out=agg_sb, in_=agg_d.ap())

            def load_x_quarter(x_tile, b, q):
                qsl = slice(q * QW, (q + 1) * QW)
                nc.sync.dma_start(
                    out=x_tile[:, :, qsl],
                    in_=x_d.ap()[b, :, qsl].rearrange("(c p) f -> p c f", p=128),
                )

            x_tiles = [x0_sb]
            for q in range(1, XQ):
                load_x_quarter(x0_sb, 0, q)

            # --- main loop ------------------------------------------------
            for b in range(BPC):
                # per-sample accumulators:
                # cols [0:OC)      sum(y) over the A half, per chunk
                # cols [OC:2*OC)   sum(y) over the B half
                # cols [2*OC:3*OC) sum(y^2)
                sums = small.tile([128, 3 * OC], f32, tag="sums")
                y_tiles = []
                ps_b3 = None
                for oc in range(OC):
                    # prefetch next sample's x between early chunks so the
                    # quarters enter the SP DMA FIFO ahead of later stores
                    if b + 1 < BPC and oc < 2:
                        if oc == 0:
                            xnext = xp.tile([128, KC, HW], f16, tag="x")
                            x_tiles.append(xnext)
                        for j in range(XQ // 2):
                            load_x_quarter(
                                x_tiles[b + 1], b + 1, (XQ // 2) * oc + j
                            )
                    x_sb = x_tiles[b]
                    osl = slice(oc * 128, (oc + 1) * 128)

                    psA = psy.tile([128, 2048], f32, tag="ps")
                    psB = psy.tile([128, 2048], f32, tag="ps")
                    y_sb = yp.tile([128, HW], f16, tag="y")
                    y_tiles.append(y_sb)

                    for lo, wdt in A_TILES:
                        for c in range(KC):
                            nc.tensor.matmul(
                                psA[:, lo : lo + wdt],
                                wt_sb[:, c, osl],
                                x_sb[:, c, lo : lo + wdt],
                                start=(c == 0),
                                stop=(c == KC - 1),
                            )
                    nc.scalar.activation(
                        out=y_sb[:, 0:HWA],
                        in_=psA[:, 0:HWA],
                        func=mybir.ActivationFunctionType.Copy,
                        accum_out=sums[:, oc : oc + 1],
                    )
                    for lo, wdt in B_TILES:
                        for c in range(KC):
                            nc.tensor.matmul(
                                psB[:, lo : lo + wdt],
                                wt_sb[:, c, osl],
                                x_sb[:, c, HWA + lo : HWA + lo + wdt],
                                start=(c == 0),
                                stop=(c == KC - 1),
                            )
                    nc.scalar.activation(
                        out=y_sb[:, HWA:HW],
                        in_=psB[:, 0:HWB],
                        func=mybir.ActivationFunctionType.Copy,
                        accum_out=sums[:, OC + oc : OC + oc + 1],
                    )
                    if oc == OC - 1:
                        ps_b3 = psB

                    # square y (head on gpsimd, tail on DVE 2x), then one
                    # DVE 4x reduce gives per-channel sum(y^2); the reduce's
                    # elementwise output is scratch
                    y2 = tp.tile([128, HW], f16, tag="y2")
                    nc.gpsimd.tensor_mul(
                        y2[:, 0:TP_COLS], y_sb[:, 0:TP_COLS], y_sb[:, 0:TP_COLS]
                    )
                    nc.vector.tensor_mul(
                        y2[:, TP_COLS:HW], y_sb[:, TP_COLS:HW], y_sb[:, TP_COLS:HW]
                    )
                    trash = tp.tile([128, HW], f16, tag="t")
                    nc.vector.tensor_scalar(
                        out=trash,
                        in0=y2,
                        scalar1=1.0,
                        scalar2=None,
                        op0=mybir.AluOpType.mult,
                        op1=mybir.AluOpType.add,
                        accum_out=sums[:, 2 * OC + oc : 2 * OC + oc + 1],
                    )

                # group-reduce + broadcast of all 12 accumulators in one tiny
                # matmul against the block-diagonal 1/(16*HW) matrix, written
                # into spare tail columns of the last chunk's B psum tile
                gps = ps_b3[:, GPS_OFF : GPS_OFF + 3 * OC]
                nc.tensor.matmul(
                    gps, agg_sb, sums, start=True, stop=True,
                    skip_group_check=True,
                )
                gs = small.tile([128, 3 * OC], f32, tag="gs")
                nc.vector.tensor_copy(out=gs, in_=gps)

                # m = mean = gsA + gsB; q = E[y^2]; var = q - m^2
                m4 = small.tile([128, OC], f32, tag="m4")
                nc.vector.tensor_add(m4, gs[:, 0:OC], gs[:, OC : 2 * OC])
                msq = small.tile([128, OC], f32, tag="msq")
                nc.vector.tensor_mul(msq, m4, m4)
                ve = small.tile([128, OC], f32, tag="ve")
                nc.vector.tensor_sub(ve, gs[:, 2 * OC : 3 * OC], msq)
                sd = small.tile([128, OC], f32, tag="sd")
                nc.scalar.activation(
                    out=sd,
                    in_=ve,
                    func=mybir.ActivationFunctionType.Sqrt,
                    bias=eps_sb,
                )
                rstd = small.tile([128, OC], f32, tag="rstd")
                nc.vector.reciprocal(rstd, sd)
                s4 = small.tile([128, OC], f32, tag="s4")
                nc.vector.tensor_mul(s4, rstd, gamma_sb)
                ms = small.tile([128, OC], f32, tag="ms")
                nc.vector.tensor_mul(ms, m4, s4)
                bv4 = small.tile([128, OC], f32, tag="bv4")
                nc.vector.tensor_sub(bv4, beta_sb, ms)

                # transform + store per chunk: affine then clamp, both DVE 4x
                for oc in range(OC):
                    osl = slice(oc * 128, (oc + 1) * 128)
                    u_sb = up.tile([128, HW], f16, tag="u")
                    nc.vector.tensor_scalar(
                        out=u_sb,
                        in0=y_tiles[oc],
                        scalar1=s4[:, oc : oc + 1],
                        scalar2=bv4[:, oc : oc + 1],
                        op0=mybir.AluOpType.mult,
                        op1=mybir.AluOpType.add,
                    )
                    f_sb = fp.tile([128, HW], f16, tag="f")
                    nc.vector.tensor_scalar(
                        out=f_sb,
                        in0=u_sb,
                        scalar1=HT_MAX,
                        scalar2=HT_MIN,
                        op0=mybir.AluOpType.min,
                        op1=mybir.AluOpType.max,
                    )
                    nc.sync.dma_start(out=out_d.ap()[b, osl, :], in_=f_sb)

    nc.compile()
    return nc


def _get_program():
    global _NC_CACHE
    if _NC_CACHE is None:
        _NC_CACHE = _build_program()
    return _NC_CACHE


def _make_in_maps(x, weight, gamma, beta):
    xr = np.ascontiguousarray(x.reshape(B, CIN, HW).astype(np.float16))
    wt = np.ascontiguousarray(weight.T.astype(np.float16))  # [CIN, COUT]
    gamma = np.ascontiguousarray(gamma, dtype=np.float32)
    beta = np.ascontiguousarray(beta, dtype=np.float32)
    agg = np.zeros((128, 128), dtype=np.float32)
    inv = 1.0 / (GSIZE * HW)
    for g in range(128 // GSIZE):
        agg[g * GSIZE : (g + 1) * GSIZE, g * GSIZE : (g + 1) * GSIZE] = inv
    return [
        {
            "x": xr[i * BPC : (i + 1) * BPC],
            "wt": wt,
            "gamma": gamma,
            "beta": beta,
            "agg": agg,
        }
        for i in range(N_CORES)
    ]


def kernel(x, weight, gamma, beta):
    x = np.asarray(x, dtype=np.float32)
    weight = np.asarray(weight, dtype=np.float32)
    assert x.shape == (B, CIN, H, W)
    nc = _get_program()
    in_maps = _make_in_maps(x, weight, gamma, beta)
    res = run_bass_kernel_spmd(nc, in_maps, core_ids=list(range(N_CORES)))
    out = np.concatenate([r["out"] for r in res.results], axis=0)
    return out.astype(np.float32).reshape(B, COUT, H, W)


# revision 5
# speedup vs baseline: 1.2803x; 1.0986x over previous
"""Trainium2 Bass kernel for: 1x1-conv GEMM + GroupNorm + HardTanh.

Reference computation (per sample b):
    y = weight @ x[b]                        # [512, 256] @ [256, 56*56]
    groupnorm over 32 groups of 16 channels  # stats over (16, 56*56)
    y = y * gamma + beta                     # per-channel affine
    out = clip(y, -2, 2)

Sharding: data-parallel over batch, 4 samples per core x 8 cores.
weight/gamma/beta replicated. No cross-core communication needed.

HBM-bandwidth-bound by design: x and the output travel as fp16
(halving DMA traffic vs fp32; fp16's 10 mantissa bits keep the
end-to-end error ~1e-3 of scale) and the matmul runs in fp16 at the
full PE rate. Engine assignment per 128-channel chunk:
  PE   : 512-col matmuls into two 4-bank PSUM tiles (2 rotating slots,
         chunk k+1 fills while chunk k drains), plus one tiny
         group-aggregation matmul per sample.
  ACT  : sole PSUM reader - Copy PSUM fp32 -> SBUF fp16 with accum_out
         giving per-channel sum(y) as a free side effect.
  Pool : squares the head columns (TensorTensor mult fp16). Its
         sum-reduce is emitted one chunk LATE on the DVE so Pool's
         latency stays off the stats critical path.
  DVE  : squares the tail columns (TT 2x mode), then in 4x mode the
         two sum(y^2) region reduces, the affine (mult s, add bv) and
         the full clamp (min 2, max -2).
Group stats are per-SAMPLE: 16 per-chunk accumulators feed one tiny PE
matmul against a replicated block-diagonal averaging matrix
(group-reduce + broadcast in one shot) then a short DVE/ACT chain.
Sample b's affine/clamp/store are software-pipelined into sample b+1's
chunk loop so the in-order DVE stream never waits on the stats chain
and output DMAs spread evenly across the sample period.
"""

import sys

sys.path.insert(0, "/opt/trn_rl_repo")

import numpy as np

import concourse.bacc as bacc
import concourse.mybir as mybir
import concourse.tile as tile
from concourse.bass_utils import run_bass_kernel_spmd

# Problem shape (hardcoded per contest contract)
B, CIN, COUT, H, W = 32, 256, 512, 56, 56
HW = H * W  # 3136
G = 32  # num groups
GSIZE = COUT // G  # 16 channels per group
EPS = 1e-5
HT_MIN, HT_MAX = -2.0, 2.0

N_CORES = 8
BPC = B // N_CORES  # samples per core = 4
KC = CIN // 128  # contraction chunks = 2
OC = COUT // 128  # output-channel chunks = 4

HWA = 2048  # first PSUM tile covers hw cols [0, 2048)
HWB = HW - HWA  # second covers [2048, 3136) = 1088 cols
A_TILES = [(t * 512, 512) for t in range(4)]
B_TILES = [(0, 512), (512, 512), (1024, 64)]
GPS_OFF = 1536  # group-stats scratch cols inside the last B psum tile

# square-pass head (gpsimd) sizes per chunk; smaller on the last chunk so
# the lag-free final head-reduce doesn't stall the stats chain
TP = [1740, 1740, 1740, 900]

XQ = 4  # x loaded in 4 column-range DMAs so matmuls start early
QW = HW // XQ  # 784

_NC_CACHE = None


def _build_program():
    f32 = mybir.dt.float32
    f16 = mybir.dt.float16

    nc = bacc.Bacc("TRN2", target_bir_lowering=False, debug=False)

    x_d = nc.dram_tensor("x", [BPC, CIN, HW], f16, kind="ExternalInput")
    wt_d = nc.dram_tensor("wt", [CIN, COUT], f16, kind="ExternalInput")
    gamma_d = nc.dram_tensor("gamma", [COUT], f32, kind="ExternalInput")
    beta_d = nc.dram_tensor("beta", [COUT], f32, kind="ExternalInput")
    agg_d = nc.dram_tensor("agg", [128, 128], f32, kind="ExternalInput")
    out_d = nc.dram_tensor("out", [BPC, COUT, HW], f16, kind="ExternalOutput")

    with tile.TileContext(nc) as tc:
        with (
            tc.tile_pool(name="singles", bufs=1) as singles,
            tc.tile_pool(name="xp", bufs=2) as xp,
            tc.tile_pool(name="yp", bufs=8) as yp,
            tc.tile_pool(name="up", bufs=3) as up,
            tc.tile_pool(name="fp", bufs=4) as fp,
            tc.tile_pool(name="tp", bufs=3) as tp,
            tc.tile_pool(name="small", bufs=2) as small,
            tc.tile_pool(name="psy", bufs=2, space="PSUM") as psy,
        ):
            # --- one-time setup -------------------------------------------
            # weights first, then the first x quarter, so matmul 1 can
            # start the moment both land; scalars ride SWDGE (gpsimd)
            wt_sb = singles.tile([128, KC, COUT], f16)
            nc.sync.dma_start(
                out=wt_sb, in_=wt_d.ap().rearrange("(c p) m -> p c m", p=128)
            )
            x0_sb = xp.tile([128, KC, HW], f16, tag="x")
            nc.sync.dma_start(
                out=x0_sb[:, :, 0:QW],
                in_=x_d.ap()[0, :, 0:QW].rearrange("(c p) f -> p c f", p=128),
            )
            gamma_sb = singles.tile([128, OC], f32)
            nc.gpsimd.dma_start(
                out=gamma_sb, in_=gamma_d.ap().rearrange("(c p) -> p c", p=128)
            )
            beta_sb = singles.tile([128, OC], f32)
            nc.gpsimd.dma_start(
                out=beta_sb, in_=beta_d.ap().rearrange("(c p) -> p c", p=128)
            )
            eps_sb = singles.tile([128, 1], f32)
            nc.vector.memset(eps_sb, EPS)
            agg_sb = singles.tile([128, 128], f32)
            nc.gpsimd.dma_start(out=agg_sb, in_=agg_d.ap())

            def load_x_quarter(x_tile, b, q):
                qsl = slice(q * QW, (q + 1) * QW)
                nc.sync.dma_start(
                    out=x_tile[:, :, qsl],
                    in_=x_d.ap()[b, :, qsl].rearrange("(c p) f -> p c f", p=128),
                )

            x_tiles = [x0_sb]
            for q in range(1, XQ):
                load_x_quarter(x0_sb, 0, q)

            # per-sample state carried across the pipeline
            y_tiles = {}  # (b, oc) -> y_sb
            y2_tiles = {}  # (b, oc) -> y2
            sums_t = {}  # b -> accumulator tile
            sb_t = {}  # b -> (s4, bv4)

            def emit_chunk(b, oc):
                """matmuls + PSUM evacuation + square + region reduces."""
                x_sb = x_tiles[b]
                osl = slice(oc * 128, (oc + 1) * 128)
                sums = sums_t[b]

                psA = psy.tile([128, 2048], f32, tag="ps")
                psB = psy.tile([128, 2048], f32, tag="ps")
                y_sb = yp.tile([128, HW], f16, tag="y")
                y_tiles[(b, oc)] = y_sb

                for lo, wdt in A_TILES:
                    for c in range(KC):
                        nc.tensor.matmul(
                            psA[:, lo : lo + wdt],
                            wt_sb[:, c, osl],
                            x_sb[:, c, lo : lo + wdt],
                            start=(c == 0),
                            stop=(c == KC - 1),
                        )
                nc.scalar.activation(
                    out=y_sb[:, 0:HWA],
                    in_=psA[:, 0:HWA],
                    func=mybir.ActivationFunctionType.Copy,
                    accum_out=sums[:, oc : oc + 1],
                )
                for lo, wdt in B_TILES:
                    for c in range(KC):
                        nc.tensor.matmul(
                            psB[:, lo : lo + wdt],
                            wt_sb[:, c, osl],
                            x_sb[:, c, HWA + lo : HWA + lo + wdt],
                            start=(c == 0),
                            stop=(c == KC - 1),
                        )
                nc.scalar.activation(
                    out=y_sb[:, HWA:HW],
                    in_=psB[:, 0:HWB],
                    func=mybir.ActivationFunctionType.Copy,
                    accum_out=sums[:, OC + oc : OC + oc + 1],
                )

                # square: gpsimd head (within the A region so it only waits
                # on the A copy), DVE 2x tail
                tp_c = TP[oc]
                y2 = tp.tile([128, HW], f16, tag="y2")
                y2_tiles[(b, oc)] = y2
                nc.gpsimd.tensor_mul(
                    y2[:, 0:tp_c], y_sb[:, 0:tp_c], y_sb[:, 0:tp_c]
                )
                nc.vector.tensor_mul(
                    y2[:, tp_c:HW], y_sb[:, tp_c:HW], y_sb[:, tp_c:HW]
                )
                # immediate tail reduce; the head reduce is emitted one
                # chunk later (sum_head) so Pool latency never stalls DVE
                trash = tp.tile([128, HW], f16, tag="t")
                nc.vector.tensor_scalar(
                    out=trash[:, tp_c:HW],
                    in0=y2[:, tp_c:HW],
                    scalar1=1.0,
                    scalar2=None,
                    op0=mybir.AluOpType.mult,
                    op1=mybir.AluOpType.add,
                    accum_out=sums[:, 3 * OC + oc : 3 * OC + oc + 1],
                )
                return psB

            def sum_head(b, oc):
                tp_c = TP[oc]
                sums = sums_t[b]
                trash = tp.tile([128, HW], f16, tag="t")
                nc.vector.tensor_scalar(
                    out=trash[:, 0:tp_c],
                    in0=y2_tiles[(b, oc)][:, 0:tp_c],
                    scalar1=1.0,
                    scalar2=None,
                    op0=mybir.AluOpType.mult,
                    op1=mybir.AluOpType.add,
                    accum_out=sums[:, 2 * OC + oc : 2 * OC + oc + 1],
                )

            def emit_chain(b, ps_b3):
                """per-sample group stats -> per-channel scale/bias."""
                sums = sums_t[b]
                gps = ps_b3[:, GPS_OFF : GPS_OFF + 4 * OC]
                nc.tensor.matmul(
                    gps, agg_sb, sums, start=True, stop=True,
                    skip_group_check=True,
                )
                gs = small.tile([128, 4 * OC], f32, tag="gs")
                nc.vector.tensor_copy(out=gs, in_=gps)
                m4 = small.tile([128, OC], f32, tag="m4")
                nc.vector.tensor_add(m4, gs[:, 0:OC], gs[:, OC : 2 * OC])
                q4 = small.tile([128, OC], f32, tag="q4")
                nc.vector.tensor_add(
                    q4, gs[:, 2 * OC : 3 * OC], gs[:, 3 * OC : 4 * OC]
                )
                msq = small.tile([128, OC], f32, tag="msq")
                nc.vector.tensor_mul(msq, m4, m4)
                ve = small.tile([128, OC], f32, tag="ve")
                nc.vector.tensor_sub(ve, q4, msq)
                sd = small.tile([128, OC], f32, tag="sd")
                nc.scalar.activation(
                    out=sd,
                    in_=ve,
                    func=mybir.ActivationFunctionType.Sqrt,
                    bias=eps_sb,
                )
                rstd = small.tile([128, OC], f32, tag="rstd")
                nc.vector.reciprocal(rstd, sd)
                s4 = small.tile([128, OC], f32, tag="s4")
                nc.vector.tensor_mul(s4, rstd, gamma_sb)
                ms = small.tile([128, OC], f32, tag="ms")
                nc.vector.tensor_mul(ms, m4, s4)
                bv4 = small.tile([128, OC], f32, tag="bv4")
                nc.vector.tensor_sub(bv4, beta_sb, ms)
                sb_t[b] = (s4, bv4)

            def emit_transform(b, oc):
                """affine + clamp (DVE 4x) + one store DMA."""
                s4, bv4 = sb_t[b]
                osl = slice(oc * 128, (oc + 1) * 128)
                u_sb = up.tile([128, HW], f16, tag="u")
                nc.vector.tensor_scalar(
                    out=u_sb,
                    in0=y_tiles.pop((b, oc)),
                    scalar1=s4[:, oc : oc + 1],
                    scalar2=bv4[:, oc : oc + 1],
                    op0=mybir.AluOpType.mult,
                    op1=mybir.AluOpType.add,
                )
                f_sb = fp.tile([128, HW], f16, tag="f")
                nc.vector.tensor_scalar(
                    out=f_sb,
                    in0=u_sb,
                    scalar1=HT_MAX,
                    scalar2=HT_MIN,
                    op0=mybir.AluOpType.min,
                    op1=mybir.AluOpType.max,
                )
                nc.sync.dma_start(out=out_d.ap()[b, osl, :], in_=f_sb)

            # --- main software-pipelined loop -----------------------------
            for b in range(BPC):
                sums_t[b] = small.tile(
                    [128, 4 * OC], f32, tag="sums", name="sums"
                )
                ps_b3 = None
                for oc in range(OC):
                    if b + 1 < BPC and oc < 2:
                        if oc == 0:
                            xnext = xp.tile([128, KC, HW], f16, tag="x")
                            x_tiles.append(xnext)
                        for j in range(XQ // 2):
                            load_x_quarter(
                                x_tiles[b + 1], b + 1, (XQ // 2) * oc + j
                            )
                    ps_b3 = emit_chunk(b, oc)
                    if oc > 0:
                        sum_head(b, oc - 1)
                    # previous sample's transform slots in here, keeping
                    # the DVE stream busy while this sample's stats build
                    if b > 0:
                        emit_transform(b - 1, oc)
                sum_head(b, OC - 1)
                emit_chain(b, ps_b3)
            for oc in range(OC):
                emit_transform(BPC - 1, oc)

    nc.compile()
    return nc


def _get_program():
    global _NC_CACHE
    if _NC_CACHE is None:
        _NC_CACHE = _build_program()
    return _NC_CACHE


def _make_in_maps(x, weight, gamma, beta):
    xr = np.ascontiguousarray(x.reshape(B, CIN, HW).astype(np.float16))
    wt = np.ascontiguousarray(weight.T.astype(np.float16))  # [CIN, COUT]
    gamma = np.ascontiguousarray(gamma, dtype=np.float32)
    beta = np.ascontiguousarray(beta, dtype=np.float32)
    agg = np.zeros((128, 128), dtype=np.float32)
    inv = 1.0 / (GSIZE * HW)
    for g in range(128 // GSIZE):
        agg[g * GSIZE : (g + 1) * GSIZE, g * GSIZE : (g + 1) * GSIZE] = inv
    return [
        {
            "x": xr[i * BPC : (i + 1) * BPC],
            "wt": wt,
            "gamma": gamma,
            "beta": beta,
            "agg": agg,
        }
        for i in range(N_CORES)
    ]


def kernel(x, weight, gamma, beta):
    x = np.asarray(x, dtype=np.float32)
    weight = np.asarray(weight, dtype=np.float32)
    assert x.shape == (B, CIN, H, W)
    nc = _get_program()
    in_maps = _make_in_maps(x, weight, gamma, beta)
    res = run_bass_kernel_spmd(nc, in_maps, core_ids=list(range(N_CORES)))
    out = np.concatenate([r["out"] for r in res.results], axis=0)
    return out.astype(np.float32).reshape(B, COUT, H, W)
